# revision 2
# baseline (speedup 1.0000x reference)
"""Trainium2 Bass kernel for nn_CustomNetwork (4-layer 4096x4096 MLP with
train-mode BatchNorm1d + ReLU per layer, batch-axis softmax at the end).

Strategy: data-parallel over the batch dim across 8 NeuronCores (512 rows
per core). Activations live in SBUF transposed (channels on partitions,
batch on the free dim) so BatchNorm stats and the batch-axis softmax are
native free-axis reductions. Matmuls run in bf16 (full-rate on the PE,
same as fp32r, but half the HBM traffic for W — which is what limited
the fp32 version: 256 MiB of W per core ~= 750us of DMA vs ~875us of PE
time). BN stats are taken from the fp32 PSUM accumulations, so the only
precision loss is the bf16 rounding of W and of the activations
(~0.1% RMS each per layer; harness gate is 2e-2).

Cross-core BatchNorm mean/var and the softmax exp-sum use AllReduce over
small per-channel vectors, chunked so the first chunk's latency hides
under the remaining matmuls.

Note: the Linear bias `b` is mathematically canceled by BatchNorm's mean
subtraction, so it is never loaded.
"""

import ml_dtypes
import numpy as np

import concourse.bacc as bacc
import concourse.mybir as mybir
import concourse.tile as tile
from concourse import bass_utils

P = 128  # SBUF partitions
D = 4096  # feature width
KT = D // P  # 32 k/n tiles
BM = 512  # per-core batch (4096 / 8 cores)
NSUP = 8  # n supertiles of 512 output channels
L = 4  # layers
N_CORES = 8
BN_EPS = 1e-5
# BN-stat allreduce chunking: first chunk issued early so its latency hides
# under the remaining matmuls; the small tail chunk is covered by the next
# layer's first k-steps.
CHUNKS = [(0, 28), (28, 32)]

F32 = mybir.dt.float32
BF16 = mybir.dt.bfloat16

_cached_nc = None


def build():
    global _cached_nc
    if _cached_nc is not None:
        return _cached_nc
    nc = bacc.Bacc("TRN2", target_bir_lowering=False, debug=False, num_devices=N_CORES)

    # x^T per core, packed [KT, P, BM] so each k-tile is one contiguous DMA
    xt = nc.dram_tensor("xt", [KT, P, BM], BF16, kind="ExternalInput")
    # W packed on host to [L, NSUP, KT, P, 512]: the (l, ns, k) weight tile
    # is 128 KiB contiguous, streamed in exact consumption order
    Wt = nc.dram_tensor("W", [L, NSUP, KT, P, 512], BF16, kind="ExternalInput")
    # gammaH/betaH are host-transposed to [L, P, KT] (channel t*128+p at
    # [l, p, t]) so the DMA runs with contiguous lines
    gamma = nc.dram_tensor("gammaH", [L, P, KT], F32, kind="ExternalInput")
    beta = nc.dram_tensor("betaH", [L, P, KT], F32, kind="ExternalInput")
    outt = nc.dram_tensor("outt", [D, BM], F32, kind="ExternalOutput")

    rg = [list(range(N_CORES))]

    with tile.TileContext(nc) as tc:
        with (
            tc.tile_pool(name="hbuf", bufs=1) as hpool,
            tc.tile_pool(name="wpool", bufs=40) as wpool,
            tc.tile_pool(name="psum", bufs=2, space="PSUM") as psum,
            tc.tile_pool(name="small", bufs=2) as small,
            tc.tile_pool(name="gb", bufs=1) as gbpool,
            tc.tile_pool(name="dram", bufs=1, space="DRAM") as dram,
        ):
            h = [
                hpool.tile([P, KT, BM], BF16, name="h_a"),
                hpool.tile([P, KT, BM], BF16, name="h_b"),
            ]
            # final-layer fp32 buffer: exp values, then normalized softmax out
            fout = hpool.tile([P, KT, BM], F32, name="fout")

            # x^T -> h[0], interleaved with layer-0/ns-0 W prefetch so the
            # first matmuls start within a few us
            w_pre = []
            for k in range(KT):
                nc.sync.dma_start(h[0][:, k, :], xt.ap()[k])
                wt = wpool.tile([P, 512], BF16, name="wt")
                nc.sync.dma_start(wt[:], Wt.ap()[0, 0, k])
                w_pre.append(wt)

            # gamma/beta for all layers, laid out [p, tile] per layer
            gam = gbpool.tile([P, L, KT], F32, name="gam")
            bet = gbpool.tile([P, L, KT], F32, name="bet")
            for l in range(L):
                nc.sync.dma_start(gam[:, l, :], gamma.ap()[l])
                nc.sync.dma_start(bet[:, l, :], beta.ap()[l])

            sumexp = small.tile([P, KT], F32, name="sumexp")

            for l in range(L):
                src = h[l % 2]
                dst = h[(l + 1) % 2]

                stat6 = small.tile([P, KT, 6], F32, name=f"stat6_{l}")
                meanvar = small.tile([P, KT, 2], F32, name=f"meanvar_{l}")

                # ---- matmul phase: out^T[n, m] = sum_k W[k, n] * h^T[k, m]
                for ns in range(NSUP):
                    ps = psum.tile([P, 4, BM], F32, name="ps")
                    for k in range(KT):
                        if l == 0 and ns == 0:
                            wt = w_pre[k]
                        else:
                            wt = wpool.tile([P, 512], BF16, name="wt")
                            nc.sync.dma_start(wt[:], Wt.ap()[l, ns, k])
                        for j in range(4):
                            nc.tensor.matmul(
                                ps[:, j, :],
                                wt[:, j * P : (j + 1) * P],
                                src[:, k, :],
                                start=(k == 0),
                                stop=(k == KT - 1),
                            )
                    for j in range(4):
                        t = ns * 4 + j
                        # pre-BN activations to SBUF in bf16; batch stats are
                        # taken from the fp32 PSUM copy for accuracy
                        nc.vector.tensor_copy(dst[:, t, :], ps[:, j, :])
                        nc.vector.bn_stats(stat6[:, t, :], ps[:, j, :])
                        nc.vector.bn_aggr(meanvar[:, t, :], stat6[:, t, :])

                # ---- BN: chunked cross-core mean / E[h^2] allreduce + apply
                for ci, (t0, t1) in enumerate(CHUNKS):
                    n = t1 - t0
                    pack = small.tile([P, 2, n], F32, name=f"pack_{l}_{ci}")
                    # pack[:,0,:] = local mean; pack[:,1,:] = var + mean^2
                    nc.vector.tensor_copy(pack[:, 0, :], meanvar[:, t0:t1, 0])
                    nc.vector.tensor_tensor(
                        pack[:, 1, :],
                        meanvar[:, t0:t1, 0],
                        meanvar[:, t0:t1, 0],
                        op=mybir.AluOpType.mult,
                    )
                    nc.vector.tensor_tensor(
                        pack[:, 1, :],
                        pack[:, 1, :],
                        meanvar[:, t0:t1, 1],
                        op=mybir.AluOpType.add,
                    )
                    ar_in = dram.tile([P, 2, n], F32, name=f"arin_{l}_{ci}")
                    ar_out = dram.tile([P, 2, n], F32, name=f"arout_{l}_{ci}")
                    nc.sync.dma_start(ar_in[:], pack[:])
                    nc.gpsimd.collective_compute(
                        "AllReduce",
                        mybir.AluOpType.add,
                        replica_groups=rg,
                        ins=[ar_in.opt()],
                        outs=[ar_out.opt()],
                    )
                    red = small.tile([P, 2, n], F32, name=f"red_{l}_{ci}")
                    nc.sync.dma_start(red[:], ar_out[:])

                    mean_g = small.tile([P, n], F32, name=f"mean_{l}_{ci}")
                    var_g = small.tile([P, n], F32, name=f"var_{l}_{ci}")
                    scale = small.tile([P, n], F32, name=f"scale_{l}_{ci}")
                    shift = small.tile([P, n], F32, name=f"shift_{l}_{ci}")
                    # global mean / E[h^2] (= sums / 8)
                    nc.vector.tensor_scalar_mul(mean_g[:], red[:, 0, :], 1.0 / N_CORES)
                    nc.vector.tensor_scalar_mul(var_g[:], red[:, 1, :], 1.0 / N_CORES)
                    # var = E[h^2] - mean^2
                    nc.vector.tensor_tensor(
                        scale[:], mean_g[:], mean_g[:], op=mybir.AluOpType.mult
                    )
                    nc.vector.tensor_sub(var_g[:], var_g[:], scale[:])
                    # scale = gamma / sqrt(var + eps); shift = beta - mean*scale
                    nc.vector.tensor_scalar_add(var_g[:], var_g[:], BN_EPS)
                    nc.scalar.activation(
                        scale[:],
                        var_g[:],
                        mybir.ActivationFunctionType.Sqrt,
                        bias=0.0,
                        scale=1.0,
                    )
                    nc.vector.reciprocal(scale[:], scale[:])
                    nc.vector.tensor_mul(scale[:], scale[:], gam[:, l, t0:t1])
                    nc.vector.tensor_tensor(
                        shift[:], mean_g[:], scale[:], op=mybir.AluOpType.mult
                    )
                    nc.vector.tensor_sub(shift[:], bet[:, l, t0:t1], shift[:])

                    # apply: h_next = relu(h_pre * scale + shift), in place on
                    # the bf16 buffer. On the final layer, fuse the softmax
                    # numerator instead: exp(relu(z)) = max(exp(z), 1); the
                    # DVE max also accumulates the per-channel sum for the
                    # denominator.
                    for i in range(n):
                        t = t0 + i
                        if l < L - 1:
                            nc.scalar.activation(
                                dst[:, t, :],
                                dst[:, t, :],
                                mybir.ActivationFunctionType.Relu,
                                bias=shift[:, i : i + 1],
                                scale=scale[:, i : i + 1],
                            )
                        else:
                            nc.scalar.activation(
                                fout[:, t, :],
                                dst[:, t, :],
                                mybir.ActivationFunctionType.Exp,
                                bias=shift[:, i : i + 1],
                                scale=scale[:, i : i + 1],
                            )
                            nc.vector.tensor_scalar(
                                fout[:, t, :],
                                fout[:, t, :],
                                1.0,
                                0.0,
                                mybir.AluOpType.max,
                                mybir.AluOpType.add,
                                accum_out=sumexp[:, t : t + 1],
                            )

                    # final layer: softmax denominator allreduce + normalize +
                    # store for this chunk. Emitting it here (between the two
                    # BN-stat allreduces) lets chunk 0's sum-allreduce run on
                    # the TOPSP while the tail matmuls are still executing.
                    if l == L - 1:
                        ar_in2 = dram.tile([P, n], F32, name=f"sarin_{ci}")
                        ar_out2 = dram.tile([P, n], F32, name=f"sarout_{ci}")
                        nc.sync.dma_start(ar_in2[:], sumexp[:, t0:t1])
                        nc.gpsimd.collective_compute(
                            "AllReduce",
                            mybir.AluOpType.add,
                            replica_groups=rg,
                            ins=[ar_in2.opt()],
                            outs=[ar_out2.opt()],
                        )
                        rsum = small.tile([P, n], F32, name=f"rsum_{ci}")
                        nc.sync.dma_start(rsum[:], ar_out2[:])
                        nc.vector.reciprocal(rsum[:], rsum[:])
                        for i in range(n):
                            t = t0 + i
                            nc.vector.tensor_scalar_mul(
                                fout[:, t, :], fout[:, t, :], rsum[:, i : i + 1]
                            )
                            nc.sync.dma_start(
                                outt.ap()[t * P : (t + 1) * P, :], fout[:, t, :]
                            )

    nc.compile()
    _cached_nc = nc
    return nc


def make_in_maps(x, W, gamma, beta):
    """Host-side shard/pack: per-core input dict list for run_bass_kernel_spmd."""
    x = np.asarray(x, dtype=np.float32)
    W = np.asarray(W, dtype=np.float32)
    gamma = np.asarray(gamma, dtype=np.float32)
    beta = np.asarray(beta, dtype=np.float32)

    # [L, D, D] -> [L, NSUP, KT, P, 512] bf16, contiguous per (l, ns, k) tile
    Wp = np.ascontiguousarray(
        W.reshape(L, KT, P, NSUP, 512).transpose(0, 3, 1, 2, 4)
    ).astype(ml_dtypes.bfloat16)
    # [L, D] -> [L, P, KT]: channel (t*128 + p) lands at [l, p, t]
    gammaH = np.ascontiguousarray(gamma.reshape(L, KT, P).transpose(0, 2, 1))
    betaH = np.ascontiguousarray(beta.reshape(L, KT, P).transpose(0, 2, 1))
    in_maps = []
    for c in range(N_CORES):
        xt_c = (
            np.ascontiguousarray(x[c * BM : (c + 1) * BM, :].T)
            .astype(ml_dtypes.bfloat16)
            .reshape(KT, P, BM)
        )
        in_maps.append({"xt": xt_c, "W": Wp, "gammaH": gammaH, "betaH": betaH})
    return in_maps


def kernel(x, W, b, gamma, beta):
    """Full (unsharded) inputs -> full [4096, 4096] softmax output."""
    del b  # canceled by BatchNorm mean subtraction
    nc = build()
    in_maps = make_in_maps(x, W, gamma, beta)
    r = bass_utils.run_bass_kernel_spmd(nc, in_maps, core_ids=list(range(N_CORES)))
    out = np.empty((N_CORES * BM, D), dtype=np.float32)
    for c in range(N_CORES):
        out[c * BM : (c + 1) * BM, :] = r.results[c]["outt"].T
    return out


# revision 3
# speedup vs baseline: 1.0220x; 1.0220x over previous
"""Trainium2 Bass kernel for nn_CustomNetwork (4-layer 4096x4096 MLP with
train-mode BatchNorm1d + ReLU per layer, batch-axis softmax at the end).

Strategy: data-parallel over the batch dim across 8 NeuronCores (512 rows
per core). Activations live in SBUF transposed (channels on partitions,
batch on the free dim) so BatchNorm stats and the batch-axis softmax are
native free-axis reductions. Matmuls run in fp16 (full PE rate, like
fp32r/bf16, but half the HBM traffic for W and ~8x less rounding error
than bf16). BN stats are taken from the fp32 PSUM accumulations.

Cross-core BatchNorm mean/var and the softmax exp-sum use AllReduce over
small per-channel vectors. Scheduling notes (from perfetto traces):
 - the PE sustains ~269 ns per 128x128x512 matmul (the 128-cycle weight
   reload between matmuls does not overlap the stream), so the kernel is
   PE-bound at ~1.08 ms of matmul; everything else must hide under it.
 - chunk-0 BN work (tiles 0..27) is emitted BETWEEN supertile 6 and
   supertile 7 so its DVE ops are not FIFO-blocked behind supertile 7's
   stats, letting the apply chain finish before the layer ends.
 - chunk-1 (supertile 7) emits bn_stats before the PSUM->SBUF casts so
   its allreduce launches ~3.5 us after the last matmul; the next layer's
   first 28 k-steps (~24 us) hide the allreduce latency (~15 us).
 - a dummy warm-up AllReduce at kernel start absorbs the ~13 us
   first-collective initialization cost.
 - the final layer issues BN-AR(chunk1) before the softmax sum-ARs so the
   long chunk-1 chain overlaps the chunk-0 exp/sum work, and tiles 0..27
   normalize + stream out while chunk 1 is still reducing.

Note: the Linear bias `b` is mathematically canceled by BatchNorm's mean
subtraction, so it is never loaded.
"""

import numpy as np

import concourse.bacc as bacc
import concourse.mybir as mybir
import concourse.tile as tile
from concourse import bass_utils

P = 128  # SBUF partitions
D = 4096  # feature width
KT = D // P  # 32 k/n tiles
BM = 512  # per-core batch (4096 / 8 cores)
NSUP = 8  # n supertiles of 512 output channels
L = 4  # layers
N_CORES = 8
BN_EPS = 1e-5
SPLIT = 28  # BN chunk boundary: chunk0 = tiles 0..27, chunk1 = 28..31

F32 = mybir.dt.float32
F16 = mybir.dt.float16

_cached_nc = None


def build():
    global _cached_nc
    if _cached_nc is not None:
        return _cached_nc
    nc = bacc.Bacc("TRN2", target_bir_lowering=False, debug=False, num_devices=N_CORES)

    # x^T per core, packed [KT, P, BM] so each k-tile is one contiguous DMA
    xt = nc.dram_tensor("xt", [KT, P, BM], F16, kind="ExternalInput")
    # W packed on host to [L, NSUP, KT, P, 512]: the (l, ns, k) weight tile
    # is 128 KiB contiguous, streamed in exact consumption order
    Wt = nc.dram_tensor("W", [L, NSUP, KT, P, 512], F16, kind="ExternalInput")
    # gammaH/betaH are host-transposed to [L, P, KT] (channel t*128+p at
    # [l, p, t]) so the DMA runs with contiguous lines
    gamma = nc.dram_tensor("gammaH", [L, P, KT], F32, kind="ExternalInput")
    beta = nc.dram_tensor("betaH", [L, P, KT], F32, kind="ExternalInput")
    outt = nc.dram_tensor("outt", [D, BM], F32, kind="ExternalOutput")

    rg = [list(range(N_CORES))]

    with tile.TileContext(nc) as tc:
        with (
            tc.tile_pool(name="hbuf", bufs=1) as hpool,
            tc.tile_pool(name="wpool", bufs=40) as wpool,
            tc.tile_pool(name="psum", bufs=2, space="PSUM") as psum,
            tc.tile_pool(name="small", bufs=2) as small,
            tc.tile_pool(name="gb", bufs=1) as gbpool,
            tc.tile_pool(name="dram", bufs=1, space="DRAM") as dram,
        ):
            h = [
                hpool.tile([P, KT, BM], F16, name="h_a"),
                hpool.tile([P, KT, BM], F16, name="h_b"),
            ]
            # final-layer fp32 buffer: exp values, then normalized softmax out
            fout = hpool.tile([P, KT, BM], F32, name="fout")

            # warm-up collective: absorbs the one-time ring/credit init so
            # layer 0's BN allreduce runs at warm latency
            warm_in = dram.tile([P, 2], F32, name="warm_in")
            warm_out = dram.tile([P, 2], F32, name="warm_out")
            nc.gpsimd.collective_compute(
                "AllReduce",
                mybir.AluOpType.add,
                replica_groups=rg,
                ins=[warm_in.opt()],
                outs=[warm_out.opt()],
            )

            # x^T -> h[0], interleaved with layer-0/ns-0 W prefetch so the
            # first matmuls start within a few us
            w_pre = []
            for k in range(KT):
                nc.sync.dma_start(h[0][:, k, :], xt.ap()[k])
                wt = wpool.tile([P, 512], F16, name="wt")
                nc.sync.dma_start(wt[:], Wt.ap()[0, 0, k])
                w_pre.append(wt)

            # gamma/beta for all layers, laid out [p, tile] per layer
            gam = gbpool.tile([P, L, KT], F32, name="gam")
            bet = gbpool.tile([P, L, KT], F32, name="bet")
            for l in range(L):
                nc.sync.dma_start(gam[:, l, :], gamma.ap()[l])
                nc.sync.dma_start(bet[:, l, :], beta.ap()[l])

            sumexp = small.tile([P, KT], F32, name="sumexp")

            def emit_supertile(l, ns, src, dst, stat6, meanvar):
                """k-sweep matmuls for one 512-channel output supertile, then
                batch stats (first, so the stat allreduce can launch) and the
                PSUM->fp16 casts."""
                ps = psum.tile([P, 4, BM], F32, name="ps")
                for k in range(KT):
                    if l == 0 and ns == 0:
                        wt = w_pre[k]
                    else:
                        wt = wpool.tile([P, 512], F16, name="wt")
                        nc.sync.dma_start(wt[:], Wt.ap()[l, ns, k])
                    for j in range(4):
                        nc.tensor.matmul(
                            ps[:, j, :],
                            wt[:, j * P : (j + 1) * P],
                            src[:, k, :],
                            start=(k == 0),
                            stop=(k == KT - 1),
                        )
                for j in range(4):
                    t = ns * 4 + j
                    nc.vector.bn_stats(stat6[:, t, :], ps[:, j, :])
                    nc.vector.bn_aggr(meanvar[:, t, :], stat6[:, t, :])
                for j in range(4):
                    t = ns * 4 + j
                    nc.vector.tensor_copy(dst[:, t, :], ps[:, j, :])

            def emit_bn_reduce(l, ci, t0, t1, meanvar):
                """Pack local mean / E[h^2] for tiles [t0,t1) and allreduce."""
                n = t1 - t0
                pack = small.tile([P, 2, n], F32, name=f"pack_{l}_{ci}")
                nc.vector.tensor_copy(pack[:, 0, :], meanvar[:, t0:t1, 0])
                nc.vector.tensor_tensor(
                    pack[:, 1, :],
                    meanvar[:, t0:t1, 0],
                    meanvar[:, t0:t1, 0],
                    op=mybir.AluOpType.mult,
                )
                nc.vector.tensor_tensor(
                    pack[:, 1, :],
                    pack[:, 1, :],
                    meanvar[:, t0:t1, 1],
                    op=mybir.AluOpType.add,
                )
                ar_in = dram.tile([P, 2, n], F32, name=f"arin_{l}_{ci}")
                ar_out = dram.tile([P, 2, n], F32, name=f"arout_{l}_{ci}")
                nc.sync.dma_start(ar_in[:], pack[:])
                nc.gpsimd.collective_compute(
                    "AllReduce",
                    mybir.AluOpType.add,
                    replica_groups=rg,
                    ins=[ar_in.opt()],
                    outs=[ar_out.opt()],
                )
                red = small.tile([P, 2, n], F32, name=f"red_{l}_{ci}")
                nc.sync.dma_start(red[:], ar_out[:])
                return red

            def emit_bn_scaleshift(l, ci, red, t0, t1):
                """scale = gamma/sqrt(var+eps), shift = beta - mean*scale."""
                n = t1 - t0
                mean_g = small.tile([P, n], F32, name=f"mean_{l}_{ci}")
                var_g = small.tile([P, n], F32, name=f"var_{l}_{ci}")
                scale = small.tile([P, n], F32, name=f"scale_{l}_{ci}")
                shift = small.tile([P, n], F32, name=f"shift_{l}_{ci}")
                nc.vector.tensor_scalar_mul(mean_g[:], red[:, 0, :], 1.0 / N_CORES)
                nc.vector.tensor_scalar_mul(var_g[:], red[:, 1, :], 1.0 / N_CORES)
                nc.vector.tensor_tensor(
                    scale[:], mean_g[:], mean_g[:], op=mybir.AluOpType.mult
                )
                nc.vector.tensor_sub(var_g[:], var_g[:], scale[:])
                nc.vector.tensor_scalar_add(var_g[:], var_g[:], BN_EPS)
                nc.scalar.activation(
                    scale[:],
                    var_g[:],
                    mybir.ActivationFunctionType.Sqrt,
                    bias=0.0,
                    scale=1.0,
                )
                nc.vector.reciprocal(scale[:], scale[:])
                nc.vector.tensor_mul(scale[:], scale[:], gam[:, l, t0:t1])
                nc.vector.tensor_tensor(
                    shift[:], mean_g[:], scale[:], op=mybir.AluOpType.mult
                )
                nc.vector.tensor_sub(shift[:], bet[:, l, t0:t1], shift[:])
                return scale, shift

            def emit_apply(l, dst, scale, shift, t0, t1):
                """BN apply + ReLU in place (middle layers), or the fused
                softmax numerator exp(relu(z)) = max(exp(z), 1) -> fout on the
                last layer. The DVE max/accum for the softmax denominator is
                emitted separately (emit_accum) to keep the Vector FIFO free."""
                for i in range(t1 - t0):
                    t = t0 + i
                    if l < L - 1:
                        nc.scalar.activation(
                            dst[:, t, :],
                            dst[:, t, :],
                            mybir.ActivationFunctionType.Relu,
                            bias=shift[:, i : i + 1],
                            scale=scale[:, i : i + 1],
                        )
                    else:
                        nc.scalar.activation(
                            fout[:, t, :],
                            dst[:, t, :],
                            mybir.ActivationFunctionType.Exp,
                            bias=shift[:, i : i + 1],
                            scale=scale[:, i : i + 1],
                        )

            def emit_accum(t0, t1):
                for t in range(t0, t1):
                    nc.vector.tensor_scalar(
                        fout[:, t, :],
                        fout[:, t, :],
                        1.0,
                        0.0,
                        mybir.AluOpType.max,
                        mybir.AluOpType.add,
                        accum_out=sumexp[:, t : t + 1],
                    )

            def emit_sum_ar(ci, t0, t1):
                n = t1 - t0
                ar_in = dram.tile([P, n], F32, name=f"sarin_{ci}")
                ar_out = dram.tile([P, n], F32, name=f"sarout_{ci}")
                nc.sync.dma_start(ar_in[:], sumexp[:, t0:t1])
                nc.gpsimd.collective_compute(
                    "AllReduce",
                    mybir.AluOpType.add,
                    replica_groups=rg,
                    ins=[ar_in.opt()],
                    outs=[ar_out.opt()],
                )
                rsum = small.tile([P, n], F32, name=f"rsum_{ci}")
                nc.sync.dma_start(rsum[:], ar_out[:])
                nc.vector.reciprocal(rsum[:], rsum[:])
                return rsum

            def emit_out(rsum, t0, t1):
                for i in range(t1 - t0):
                    t = t0 + i
                    nc.vector.tensor_scalar_mul(
                        fout[:, t, :], fout[:, t, :], rsum[:, i : i + 1]
                    )
                    nc.sync.dma_start(outt.ap()[t * P : (t + 1) * P, :], fout[:, t, :])

            for l in range(L):
                src = h[l % 2]
                dst = h[(l + 1) % 2]
                stat6 = small.tile([P, KT, 6], F32, name=f"stat6_{l}")
                meanvar = small.tile([P, KT, 2], F32, name=f"meanvar_{l}")

                for ns in range(NSUP - 1):
                    emit_supertile(l, ns, src, dst, stat6, meanvar)

                # chunk 0 (tiles 0..SPLIT-1): reduce + apply while supertile 7
                # is still on the PE
                red0 = emit_bn_reduce(l, 0, 0, SPLIT, meanvar)
                sc0, sh0 = emit_bn_scaleshift(l, 0, red0, 0, SPLIT)
                emit_apply(l, dst, sc0, sh0, 0, SPLIT)

                emit_supertile(l, NSUP - 1, src, dst, stat6, meanvar)

                # chunk 1 (tiles SPLIT..31): allreduce launches right after
                # supertile 7's stats
                red1 = emit_bn_reduce(l, 1, SPLIT, KT, meanvar)

                if l == L - 1:
                    # softmax denominator for chunk 0 (queued on DVE after
                    # supertile 7's stats so they don't block the BN AR)
                    emit_accum(0, SPLIT)
                    rsum0 = emit_sum_ar(0, 0, SPLIT)

                sc1, sh1 = emit_bn_scaleshift(l, 1, red1, SPLIT, KT)
                emit_apply(l, dst, sc1, sh1, SPLIT, KT)

                if l == L - 1:
                    emit_accum(SPLIT, KT)
                    rsum1 = emit_sum_ar(1, SPLIT, KT)
                    emit_out(rsum0, 0, SPLIT)
                    emit_out(rsum1, SPLIT, KT)

    nc.compile()
    _cached_nc = nc
    return nc


def make_in_maps(x, W, gamma, beta):
    """Host-side shard/pack: per-core input dict list for run_bass_kernel_spmd."""
    x = np.asarray(x, dtype=np.float32)
    W = np.asarray(W, dtype=np.float32)
    gamma = np.asarray(gamma, dtype=np.float32)
    beta = np.asarray(beta, dtype=np.float32)

    # [L, D, D] -> [L, NSUP, KT, P, 512] fp16, contiguous per (l, ns, k) tile
    Wp = np.ascontiguousarray(
        W.reshape(L, KT, P, NSUP, 512).transpose(0, 3, 1, 2, 4)
    ).astype(np.float16)
    # [L, D] -> [L, P, KT]: channel (t*128 + p) lands at [l, p, t]
    gammaH = np.ascontiguousarray(gamma.reshape(L, KT, P).transpose(0, 2, 1))
    betaH = np.ascontiguousarray(beta.reshape(L, KT, P).transpose(0, 2, 1))
    in_maps = []
    for c in range(N_CORES):
        xt_c = (
            np.ascontiguousarray(x[c * BM : (c + 1) * BM, :].T)
            .astype(np.float16)
            .reshape(KT, P, BM)
        )
        in_maps.append({"xt": xt_c, "W": Wp, "gammaH": gammaH, "betaH": betaH})
    return in_maps


def kernel(x, W, b, gamma, beta):
    """Full (unsharded) inputs -> full [4096, 4096] softmax output."""
    del b  # canceled by BatchNorm mean subtraction
    nc = build()
    in_maps = make_in_maps(x, W, gamma, beta)
    r = bass_utils.run_bass_kernel_spmd(nc, in_maps, core_ids=list(range(N_CORES)))
    out = np.empty((N_CORES * BM, D), dtype=np.float32)
    for c in range(N_CORES):
        out[c * BM : (c + 1) * BM, :] = r.results[c]["outt"].T
    return out


# revision 4
# speedup vs baseline: 1.0498x; 1.0272x over previous
"""Trainium2 Bass kernel for nn_CustomNetwork (4-layer 4096x4096 MLP with
train-mode BatchNorm1d + ReLU per layer, batch-axis softmax at the end).

Strategy: data-parallel over the batch dim across 8 NeuronCores (512 rows
per core). Activations live in SBUF transposed (channels on partitions,
batch on the free dim) so BatchNorm stats and the batch-axis softmax are
native free-axis reductions. Matmuls run in fp16 (full PE rate, like
fp32r/bf16, but half the HBM traffic for W and ~8x less rounding error
than bf16). BN stats are taken from the fp32 PSUM accumulations.

Cross-core BatchNorm mean/var and the softmax exp-sum use AllReduce over
small per-channel vectors. Scheduling notes (from perfetto traces):
 - the PE sustains ~265 ns per 128x128x512 matmul (the 128-cycle weight
   reload between matmuls cannot overlap the stream on TRN2), so the
   kernel is PE-bound at ~1.09 ms of matmul; everything else must hide
   under it.
 - BN chunk 0 is tiles 0..23 (supertiles 0..5), reduced+applied while
   supertiles 6-7 are on the PE; chunk 1 (supertiles 6-7) reduces right
   after the last matmul and its apply must beat the next layer's k=24
   step (~20 us of runway).
 - the Tile scheduler orders each engine's stream by simulated readiness,
   which puts the chunk-0 scale/shift after supertile 6/7's stats (its
   AllReduce input arrives "late" in sim). add_dep_helper pins the order
   so the apply chain runs as soon as the real (fast, ~9 us) AllReduce
   lands; otherwise every layer boundary pays ~10 us.
 - a dummy warm-up AllReduce at kernel start absorbs the ~13 us
   first-collective initialization cost.
 - the final layer fully processes chunk 0 (BN -> exp -> sum-AllReduce ->
   normalize -> DMA out) while supertiles 6-7 still run, so only the
   8-tile chunk-1 chain remains after the last matmul.

Note: the Linear bias `b` is mathematically canceled by BatchNorm's mean
subtraction, so it is never loaded.
"""

import numpy as np

import concourse.bacc as bacc
import concourse.mybir as mybir
import concourse.tile as tile
from concourse import bass_utils
from concourse.tile import add_dep_helper

P = 128  # SBUF partitions
D = 4096  # feature width
KT = D // P  # 32 k/n tiles
BM = 512  # per-core batch (4096 / 8 cores)
NSUP = 8  # n supertiles of 512 output channels
L = 4  # layers
N_CORES = 8
BN_EPS = 1e-5
SPLIT = 24  # BN chunk boundary: chunk0 = tiles 0..23, chunk1 = 24..31

F32 = mybir.dt.float32
F16 = mybir.dt.float16

_cached_nc = None


def _ins(x):
    return getattr(x, "ins", x)


def build():
    global _cached_nc
    if _cached_nc is not None:
        return _cached_nc
    nc = bacc.Bacc("TRN2", target_bir_lowering=False, debug=False, num_devices=N_CORES)

    # x^T per core, packed [KT, P, BM] so each k-tile is one contiguous DMA
    xt = nc.dram_tensor("xt", [KT, P, BM], F16, kind="ExternalInput")
    # W packed on host to [L, NSUP, KT, P, 512]: the (l, ns, k) weight tile
    # is 128 KiB contiguous, streamed in exact consumption order
    Wt = nc.dram_tensor("W", [L, NSUP, KT, P, 512], F16, kind="ExternalInput")
    # gammaH/betaH are host-transposed to [L, P, KT] (channel t*128+p at
    # [l, p, t]) so the DMA runs with contiguous lines
    gamma = nc.dram_tensor("gammaH", [L, P, KT], F32, kind="ExternalInput")
    beta = nc.dram_tensor("betaH", [L, P, KT], F32, kind="ExternalInput")
    outt = nc.dram_tensor("outt", [D, BM], F32, kind="ExternalOutput")

    rg = [list(range(N_CORES))]

    with tile.TileContext(nc) as tc:
        with (
            tc.tile_pool(name="hbuf", bufs=1) as hpool,
            tc.tile_pool(name="wpool", bufs=40) as wpool,
            tc.tile_pool(name="psum", bufs=2, space="PSUM") as psum,
            tc.tile_pool(name="small", bufs=2) as small,
            tc.tile_pool(name="gb", bufs=1) as gbpool,
            tc.tile_pool(name="dram", bufs=1, space="DRAM") as dram,
        ):
            h = [
                hpool.tile([P, KT, BM], F16, name="h_a"),
                hpool.tile([P, KT, BM], F16, name="h_b"),
            ]
            # final-layer fp32 buffer: exp values, then normalized softmax out
            fout = hpool.tile([P, KT, BM], F32, name="fout")

            # warm-up collective: absorbs the one-time ring/credit init so
            # layer 0's BN allreduce runs at warm latency
            warm_in = dram.tile([P, 2], F32, name="warm_in")
            warm_out = dram.tile([P, 2], F32, name="warm_out")
            nc.gpsimd.collective_compute(
                "AllReduce",
                mybir.AluOpType.add,
                replica_groups=rg,
                ins=[warm_in.opt()],
                outs=[warm_out.opt()],
            )

            # x^T -> h[0], interleaved with layer-0/ns-0 W prefetch so the
            # first matmuls start within a few us
            w_pre = []
            for k in range(KT):
                nc.sync.dma_start(h[0][:, k, :], xt.ap()[k])
                wt = wpool.tile([P, 512], F16, name="wt")
                nc.sync.dma_start(wt[:], Wt.ap()[0, 0, k])
                w_pre.append(wt)

            # gamma/beta for all layers, laid out [p, tile] per layer
            gam = gbpool.tile([P, L, KT], F32, name="gam")
            bet = gbpool.tile([P, L, KT], F32, name="bet")
            for l in range(L):
                nc.sync.dma_start(gam[:, l, :], gamma.ap()[l])
                nc.sync.dma_start(bet[:, l, :], beta.ap()[l])

            sumexp = small.tile([P, KT], F32, name="sumexp")

            def emit_supertile(l, ns, src, dst, stat6, meanvar):
                """k-sweep matmuls for one 512-channel output supertile, then
                batch stats (first, so the stat allreduce can launch) and the
                PSUM->fp16 casts. Returns the first bn_stats instruction."""
                ps = psum.tile([P, 4, BM], F32, name="ps")
                for k in range(KT):
                    if l == 0 and ns == 0:
                        wt = w_pre[k]
                    else:
                        wt = wpool.tile([P, 512], F16, name="wt")
                        nc.sync.dma_start(wt[:], Wt.ap()[l, ns, k])
                    for j in range(4):
                        nc.tensor.matmul(
                            ps[:, j, :],
                            wt[:, j * P : (j + 1) * P],
                            src[:, k, :],
                            start=(k == 0),
                            stop=(k == KT - 1),
                        )
                first_stats = None
                for j in range(4):
                    t = ns * 4 + j
                    st = nc.vector.bn_stats(stat6[:, t, :], ps[:, j, :])
                    if first_stats is None:
                        first_stats = st
                    nc.vector.bn_aggr(meanvar[:, t, :], stat6[:, t, :])
                for j in range(4):
                    t = ns * 4 + j
                    nc.vector.tensor_copy(dst[:, t, :], ps[:, j, :])
                return first_stats

            def emit_bn_reduce(l, ci, t0, t1, meanvar):
                """Pack local mean / E[h^2] for tiles [t0,t1) and allreduce."""
                n = t1 - t0
                pack = small.tile([P, 2, n], F32, name=f"pack_{l}_{ci}")
                nc.vector.tensor_copy(pack[:, 0, :], meanvar[:, t0:t1, 0])
                nc.vector.tensor_tensor(
                    pack[:, 1, :],
                    meanvar[:, t0:t1, 0],
                    meanvar[:, t0:t1, 0],
                    op=mybir.AluOpType.mult,
                )
                nc.vector.tensor_tensor(
                    pack[:, 1, :],
                    pack[:, 1, :],
                    meanvar[:, t0:t1, 1],
                    op=mybir.AluOpType.add,
                )
                ar_in = dram.tile([P, 2, n], F32, name=f"arin_{l}_{ci}")
                ar_out = dram.tile([P, 2, n], F32, name=f"arout_{l}_{ci}")
                nc.sync.dma_start(ar_in[:], pack[:])
                nc.gpsimd.collective_compute(
                    "AllReduce",
                    mybir.AluOpType.add,
                    replica_groups=rg,
                    ins=[ar_in.opt()],
                    outs=[ar_out.opt()],
                )
                red = small.tile([P, 2, n], F32, name=f"red_{l}_{ci}")
                nc.sync.dma_start(red[:], ar_out[:])
                return red

            def emit_bn_scaleshift(l, ci, red, t0, t1):
                """scale = gamma/sqrt(var+eps), shift = beta - mean*scale.
                Returns (scale, shift, last_instruction)."""
                n = t1 - t0
                mean_g = small.tile([P, n], F32, name=f"mean_{l}_{ci}")
                var_g = small.tile([P, n], F32, name=f"var_{l}_{ci}")
                scale = small.tile([P, n], F32, name=f"scale_{l}_{ci}")
                shift = small.tile([P, n], F32, name=f"shift_{l}_{ci}")
                nc.vector.tensor_scalar_mul(mean_g[:], red[:, 0, :], 1.0 / N_CORES)
                nc.vector.tensor_scalar_mul(var_g[:], red[:, 1, :], 1.0 / N_CORES)
                nc.vector.tensor_tensor(
                    scale[:], mean_g[:], mean_g[:], op=mybir.AluOpType.mult
                )
                nc.vector.tensor_sub(var_g[:], var_g[:], scale[:])
                nc.vector.tensor_scalar_add(var_g[:], var_g[:], BN_EPS)
                nc.scalar.activation(
                    scale[:],
                    var_g[:],
                    mybir.ActivationFunctionType.Sqrt,
                    bias=0.0,
                    scale=1.0,
                )
                nc.vector.reciprocal(scale[:], scale[:])
                nc.vector.tensor_mul(scale[:], scale[:], gam[:, l, t0:t1])
                nc.vector.tensor_tensor(
                    shift[:], mean_g[:], scale[:], op=mybir.AluOpType.mult
                )
                last = nc.vector.tensor_sub(shift[:], bet[:, l, t0:t1], shift[:])
                return scale, shift, last

            def emit_apply(l, dst, scale, shift, t0, t1):
                """BN apply + ReLU in place (middle layers), or the fused
                softmax numerator exp(relu(z)) = max(exp(z), 1) -> fout on the
                last layer (the max also runs per-channel sum accumulation,
                emitted via emit_accum)."""
                for i in range(t1 - t0):
                    t = t0 + i
                    if l < L - 1:
                        nc.scalar.activation(
                            dst[:, t, :],
                            dst[:, t, :],
                            mybir.ActivationFunctionType.Relu,
                            bias=shift[:, i : i + 1],
                            scale=scale[:, i : i + 1],
                        )
                    else:
                        nc.scalar.activation(
                            fout[:, t, :],
                            dst[:, t, :],
                            mybir.ActivationFunctionType.Exp,
                            bias=shift[:, i : i + 1],
                            scale=scale[:, i : i + 1],
                        )

            def emit_accum(t0, t1):
                first = None
                for t in range(t0, t1):
                    ts = nc.vector.tensor_scalar(
                        fout[:, t, :],
                        fout[:, t, :],
                        1.0,
                        0.0,
                        mybir.AluOpType.max,
                        mybir.AluOpType.add,
                        accum_out=sumexp[:, t : t + 1],
                    )
                    if first is None:
                        first = ts
                return first

            def emit_sum_ar(ci, t0, t1):
                n = t1 - t0
                ar_in = dram.tile([P, n], F32, name=f"sarin_{ci}")
                ar_out = dram.tile([P, n], F32, name=f"sarout_{ci}")
                nc.sync.dma_start(ar_in[:], sumexp[:, t0:t1])
                nc.gpsimd.collective_compute(
                    "AllReduce",
                    mybir.AluOpType.add,
                    replica_groups=rg,
                    ins=[ar_in.opt()],
                    outs=[ar_out.opt()],
                )
                rsum = small.tile([P, n], F32, name=f"rsum_{ci}")
                nc.sync.dma_start(rsum[:], ar_out[:])
                nc.vector.reciprocal(rsum[:], rsum[:])
                return rsum

            def emit_out(rsum, t0, t1):
                for i in range(t1 - t0):
                    t = t0 + i
                    nc.vector.tensor_scalar_mul(
                        fout[:, t, :], fout[:, t, :], rsum[:, i : i + 1]
                    )
                    nc.sync.dma_start(outt.ap()[t * P : (t + 1) * P, :], fout[:, t, :])

            for l in range(L):
                src = h[l % 2]
                dst = h[(l + 1) % 2]
                stat6 = small.tile([P, KT, 6], F32, name=f"stat6_{l}")
                meanvar = small.tile([P, KT, 2], F32, name=f"meanvar_{l}")

                for ns in range(6):
                    emit_supertile(l, ns, src, dst, stat6, meanvar)

                # chunk 0 (tiles 0..23): reduce + apply while supertiles 6-7
                # are on the PE; on the last layer the full softmax chain for
                # these tiles (sum-allreduce, normalize, DMA out) also hides
                red0 = emit_bn_reduce(l, 0, 0, SPLIT, meanvar)
                sc0, sh0, ss0_last = emit_bn_scaleshift(l, 0, red0, 0, SPLIT)
                emit_apply(l, dst, sc0, sh0, 0, SPLIT)
                if l == L - 1:
                    emit_accum(0, SPLIT)
                    rsum0 = emit_sum_ar(0, 0, SPLIT)
                    emit_out(rsum0, 0, SPLIT)

                s6_stats = emit_supertile(l, 6, src, dst, stat6, meanvar)
                # pin the DVE order: chunk-0 scale/shift before supertile 6's
                # stats, so the apply chain starts when the real AR lands
                add_dep_helper(
                    _ins(s6_stats),
                    _ins(ss0_last),
                    sync=False,
                    reason="chunk0 scale/shift ahead of s6 stats on DVE",
                )
                emit_supertile(l, 7, src, dst, stat6, meanvar)

                # chunk 1 (tiles 24..31): allreduce launches right after
                # supertile 7's stats
                red1 = emit_bn_reduce(l, 1, SPLIT, KT, meanvar)
                sc1, sh1, _ = emit_bn_scaleshift(l, 1, red1, SPLIT, KT)
                emit_apply(l, dst, sc1, sh1, SPLIT, KT)

                if l == L - 1:
                    emit_accum(SPLIT, KT)
                    rsum1 = emit_sum_ar(1, SPLIT, KT)
                    emit_out(rsum1, SPLIT, KT)

    nc.compile()
    _cached_nc = nc
    return nc


def make_in_maps(x, W, gamma, beta):
    """Host-side shard/pack: per-core input dict list for run_bass_kernel_spmd."""
    x = np.asarray(x, dtype=np.float32)
    W = np.asarray(W, dtype=np.float32)
    gamma = np.asarray(gamma, dtype=np.float32)
    beta = np.asarray(beta, dtype=np.float32)

    # [L, D, D] -> [L, NSUP, KT, P, 512] fp16, contiguous per (l, ns, k) tile
    Wp = np.ascontiguousarray(
        W.reshape(L, KT, P, NSUP, 512).transpose(0, 3, 1, 2, 4)
    ).astype(np.float16)
    # [L, D] -> [L, P, KT]: channel (t*128 + p) lands at [l, p, t]
    gammaH = np.ascontiguousarray(gamma.reshape(L, KT, P).transpose(0, 2, 1))
    betaH = np.ascontiguousarray(beta.reshape(L, KT, P).transpose(0, 2, 1))
    in_maps = []
    for c in range(N_CORES):
        xt_c = (
            np.ascontiguousarray(x[c * BM : (c + 1) * BM, :].T)
            .astype(np.float16)
            .reshape(KT, P, BM)
        )
        in_maps.append({"xt": xt_c, "W": Wp, "gammaH": gammaH, "betaH": betaH})
    return in_maps


def kernel(x, W, b, gamma, beta):
    """Full (unsharded) inputs -> full [4096, 4096] softmax output."""
    del b  # canceled by BatchNorm mean subtraction
    nc = build()
    in_maps = make_in_maps(x, W, gamma, beta)
    r = bass_utils.run_bass_kernel_spmd(nc, in_maps, core_ids=list(range(N_CORES)))
    out = np.empty((N_CORES * BM, D), dtype=np.float32)
    for c in range(N_CORES):
        out[c * BM : (c + 1) * BM, :] = r.results[c]["outt"].T
    return out
